# revision 30
# baseline (speedup 1.0000x reference)
"""Coattention kernel v3 for Trainium2 (Bass/Tile), data-parallel batch on 8 cores.

v3 vs v2: (a) all PE tile-transposes replaced by DMA-XBAR slab transposes (one
dma_start(transpose=True) per block-row, bf16, eager per-m as producer tiles
land) -- the PE runs ONLY the 13 real matmuls; (b) spine (mm5/9/13) dropped
from f32r to bf16 (hw relerr 1.37e-2, budget 2e-2), removing the f32r Ws/Wx
weight copies and the f32r V slab; (c) transposed-output restructure: coT/spT/
saT are computed directly and co/sp/sa derived by transpose, so no matmul ever
waits on a freshly-transposed operand; (d) A2/A3 softmax normalization folded
into the next matmul's psum evacuation as a per-partition scale.
"""
import numpy as np
import ml_dtypes

B = 32
D = 768
P = 128
NT = D // P  # 6
N_CORES = 8
NB = B // N_CORES

BF = ml_dtypes.bfloat16

_cache = {}


def _build(nb, repeat=1, hw_loop=0):
    import concourse.bass as bass
    import concourse.mybir as mybir
    import concourse.tile as tile
    from concourse import bacc
    from contextlib import ExitStack, nullcontext

    f32 = mybir.dt.float32
    f32r = mybir.dt.float32r
    bf16 = mybir.dt.bfloat16
    Exp = mybir.ActivationFunctionType.Exp
    Copy = mybir.ActivationFunctionType.Copy

    nc = bacc.Bacc("TRN2", target_bir_lowering=False, debug=False)

    L_d = nc.dram_tensor("L", [nb, NT, P, D], f32r, kind="ExternalInput").ap()
    I_d = nc.dram_tensor("I", [nb, NT, P, D], f32r, kind="ExternalInput").ap()
    wct_d = nc.dram_tensor("wct", [D, D], f32r, kind="ExternalInput").ap()
    wcb_d = nc.dram_tensor("wcb", [D, D], bf16, kind="ExternalInput").ap()
    wsb_d = nc.dram_tensor("wsb", [D, D], bf16, kind="ExternalInput").ap()
    wxb_d = nc.dram_tensor("wxb", [D, D], bf16, kind="ExternalInput").ap()
    out_d = nc.dram_tensor("out", [nb, D, D], f32, kind="ExternalOutput").ap()

    NH = ((0, 512), (512, 768))  # psum-bank-aligned halves of the free dim

    with tile.TileContext(nc) as tc, ExitStack() as ctx:
        sb = ctx.enter_context(tc.tile_pool(name="sb", bufs=1))
        p_ltt = ctx.enter_context(tc.tile_pool(name="p_ltt", bufs=12))
        p_sm = ctx.enter_context(tc.tile_pool(name="p_sm", bufs=2))
        p_tiny = ctx.enter_context(tc.tile_pool(name="p_tiny", bufs=24))
        ps = ctx.enter_context(tc.tile_pool(name="ps", bufs=3, space="PSUM"))

        # --- weights (slab layout [P, NT, D]: wt[:, k] = W^T[kP:(k+1)P, :]) ---
        w_sb = {}
        for wname, wd, dt in (("wct", wct_d, f32r), ("wcb", wcb_d, bf16),
                              ("wsb", wsb_d, bf16), ("wxb", wxb_d, bf16)):
            wt = sb.tile([P, NT, D], dt, tag="w_" + wname)
            for k in range(NT):
                nc.sync.dma_start(wt[:, k], wd[k * P:(k + 1) * P, :])
            w_sb[wname] = wt

        def mm_stat(stat, mov, out_psum_tag="mmout", halfmajor_m0=False):
            """psum[m] = sum_e stat[:,e,mP:(m+1)P]^T @ mov[:,e,:]  (= STAT^T@MOV
            where STAT/MOV are the slab-encoded [D,D] objects).
            halfmajor_m0: finish the full [0:512] accumulation before touching
            [512:768] for m==0 -- delays the first read of mov's last column
            blocks (use when mov is a freshly DMA-transposed slab)."""
            for m in range(NT):
                pt = ps.tile([P, D], f32, tag=out_psum_tag)
                if halfmajor_m0 and m == 0:
                    for n0, n1 in NH:
                        for e in range(NT):
                            nc.tensor.matmul(pt[:, n0:n1],
                                             stat[:, e, m * P:(m + 1) * P],
                                             mov[:, e, n0:n1],
                                             start=(e == 0), stop=(e == NT - 1))
                else:
                    for e in range(NT):
                        for n0, n1 in NH:
                            nc.tensor.matmul(pt[:, n0:n1],
                                             stat[:, e, m * P:(m + 1) * P],
                                             mov[:, e, n0:n1],
                                             start=(e == 0), stop=(e == NT - 1))
                yield m, pt

        def softmax(dst, m, pt):
            sums = p_tiny.tile([P, 1], f32, tag="sums")
            nc.scalar.activation(dst[:, m], pt[:], Exp, accum_out=sums[:])
            rec = p_tiny.tile([P, 1], f32, tag="rec")
            nc.vector.reciprocal(rec[:], sums[:])
            nc.scalar.activation(dst[:, m], dst[:, m], Copy, scale=rec[:, 0:1])

        def lin_in(src_d, b):
            """yield m, psum(SRC @ Wc^T); stationary tiles DMA'd from host
            pre-transposed layout."""
            for m in range(NT):
                ltt = p_ltt.tile([P, D], f32r, tag="ltt")
                nc.sync.dma_start(ltt[:], src_d[b, m])
                pt = ps.tile([P, D], f32, tag="mmout")
                for e in range(NT):
                    for n0, n1 in NH:
                        nc.tensor.matmul(pt[:, n0:n1],
                                         ltt[:, e * P:(e + 1) * P],
                                         w_sb["wct"][:, e, n0:n1],
                                         start=(e == 0), stop=(e == NT - 1))
                yield m, pt

        def dma_T(src, m, dst):
            """dst slab ([P,NT,D], = SRC^T) block-column m <- XBAR-transpose of
            src slab block-row m.  dst[p, e, mP+b] = src[b, m, eP+p].
            MUST NOT share a queue with concurrent normal DMAs (XBAR corrupts
            even partitions, hw-verified in mini2.py): transposes go on the
            Activation hwdge queue, normal DMAs stay on nc.sync (SP)."""
            nc.scalar.dma_start(dst[:, :, m * P:(m + 1) * P], src[:, m],
                                transpose=True)

        loop_cm = tc.For_i(0, hw_loop, 1) if hw_loop else nullcontext()
        with loop_cm:
         for _r in range(repeat):
          for b in range(nb):
            t_io = sb.tile([P, NT, D], bf16, tag="io")
            t_G2 = sb.tile([P, NT, D], bf16, tag="G2", bufs=2)
            for m, pt in lin_in(I_d, b):               # mm2: io' = I@Wc^T
                nc.vector.tensor_copy(t_io[:, m], pt[:])
                dma_T(t_io, m, t_G2)                   # G2 = io'^T

            t_lo = sb.tile([P, NT, D], bf16, tag="lo")
            for m, pt in lin_in(L_d, b):               # mm1: lo' = L@Wc^T
                nc.vector.tensor_copy(t_lo[:, m], pt[:])

            t_A = sb.tile([P, NT, D], bf16, tag="A")
            for m, pt in mm_stat(t_lo, t_G2):          # mm3: S1 = lo'^T@io'^T
                softmax(t_A, m, pt)

            t_V = sb.tile([P, NT, D], bf16, tag="V")
            for m, pt in mm_stat(t_io, t_A):           # mm4: co1 = io'^T@A1
                nc.any.tensor_copy(t_V[:, m], pt[:])

            # transposed-output restructure: compute coT/spT/saT DIRECTLY (the
            # operands their immediate consumers need), and produce co/sp/sa by
            # XBAR transpose -- those are consumed 2+ steps later, so the DMA
            # latency is fully hidden (no freshly-transposed mov stall).
            t_co = sb.tile([P, NT, D], bf16, tag="co")
            t_coT = sb.tile([P, NT, D], bf16, tag="coT")
            for m, pt in mm_stat(w_sb["wcb"], t_V):    # mm5: coT = Wc@co1+lo'
                nc.vector.tensor_add(t_coT[:, m], pt[:], t_lo[:, m])
                dma_T(t_coT, m, t_co)

            t_sp = sb.tile([P, NT, D], bf16, tag="sp")
            t_spT = sb.tile([P, NT, D], bf16, tag="spT")
            for m, pt in mm_stat(w_sb["wsb"], t_coT):  # mm6: spT = Ws@co^T
                nc.vector.tensor_copy(t_spT[:, m], pt[:])
                dma_T(t_spT, m, t_sp)

            # A2/A3: exp only; the 1/rowsum scale rides the NEXT matmul's psum
            # evacuation as a per-output-partition activation scale.
            t_A = sb.tile([P, NT, D], bf16, tag="A")
            t_AT = sb.tile([P, NT, D], bf16, tag="AT")
            rec2 = []
            for m, pt in mm_stat(t_sp, t_spT):         # mm7: S2 = sp^T@sp^T
                sums = p_tiny.tile([P, 1], f32, tag="sums")
                nc.scalar.activation(t_A[:, m], pt[:], Exp, accum_out=sums[:])
                rec = p_tiny.tile([P, 1], f32, tag="rec")
                nc.vector.reciprocal(rec[:], sums[:])
                rec2.append(rec)
                dma_T(t_A, m, t_AT)                    # E2^T

            t_V = sb.tile([P, NT, D], bf16, tag="V")
            for m, pt in mm_stat(t_AT, t_co):          # mm8: sa1 = diag(rec)E2@co
                nc.scalar.activation(t_V[:, m], pt[:], Copy,
                                     scale=rec2[m][:, 0:1])

            t_sa = sb.tile([P, NT, D], bf16, tag="sa")
            t_saT = sb.tile([P, NT, D], bf16, tag="saT")
            for m, pt in mm_stat(w_sb["wsb"], t_V):    # mm9: saT = Ws@sa1+coT
                nc.vector.tensor_add(t_saT[:, m], pt[:], t_coT[:, m])
                dma_T(t_saT, m, t_sa)

            t_xp = sb.tile([P, NT, D], bf16, tag="sp")
            for m, pt in mm_stat(t_saT, w_sb["wxb"]):  # mm10: xp = sa@WxT
                nc.any.tensor_copy(t_xp[:, m], pt[:])

            t_A = sb.tile([P, NT, D], bf16, tag="A")
            t_AT = sb.tile([P, NT, D], bf16, tag="AT")
            rec3 = []
            for m, pt in mm_stat(t_xp, t_G2):          # mm11: S3 = xp^T@io'^T
                sums = p_tiny.tile([P, 1], f32, tag="sums")
                nc.scalar.activation(t_A[:, m], pt[:], Exp, accum_out=sums[:])
                rec = p_tiny.tile([P, 1], f32, tag="rec")
                nc.vector.reciprocal(rec[:], sums[:])
                rec3.append(rec)
                dma_T(t_A, m, t_AT)                    # E3^T

            t_V = sb.tile([P, NT, D], bf16, tag="V")
            for m, pt in mm_stat(t_AT, t_G2):          # mm12: xa1 = diag(rec)E3@ioT
                nc.scalar.activation(t_V[:, m], pt[:], Copy,
                                     scale=rec3[m][:, 0:1])

            for m, pt in mm_stat(t_V, w_sb["wxb"]):    # mm13: out = xa1^T@WxT+sa
                osl = p_sm.tile([P, D], f32, tag="outsl")
                nc.any.tensor_add(osl[:], pt[:], t_sa[:, m])
                nc.sync.dma_start(out_d[b, m * P:(m + 1) * P, :], osl[:])

    nc.finalize()
    return nc


def _get_program(nb, repeat=1, hw_loop=0):
    key = (nb, repeat, hw_loop)
    if key not in _cache:
        _cache[key] = _build(nb, repeat, hw_loop)
    return _cache[key]


def _round_f32r(x):
    xb = np.ascontiguousarray(x, dtype=np.float32).view(np.uint32)
    lsb = (xb >> np.uint32(12)) & np.uint32(1)
    r = (xb + np.uint32(0x7FF) + lsb) & np.uint32(0xFFFFF000)
    return r.view(np.float32)


def kernel(language_output, image_output, Wc, bc, Ws, bs, Wx, bx,
           _n_cores=N_CORES, _nb=None, _repeat=1, _hw_loop=0):
    from concourse import bass_utils

    L0 = np.asarray(language_output, dtype=np.float32)
    I0 = np.asarray(image_output, dtype=np.float32)
    nbat = L0.shape[0]
    # stationary layout: L3[b, m, p, e*128+q] = X[b, m*128+q, e*128+p]
    L = _round_f32r(np.ascontiguousarray(
        L0.reshape(nbat, NT, P, NT, P).transpose(0, 1, 4, 3, 2)
        .reshape(nbat, NT, P, D)))
    I = _round_f32r(np.ascontiguousarray(
        I0.reshape(nbat, NT, P, NT, P).transpose(0, 1, 4, 3, 2)
        .reshape(nbat, NT, P, D)))
    wct = _round_f32r(np.asarray(Wc, dtype=np.float32).T)
    wcb = np.ascontiguousarray(np.asarray(Wc, dtype=np.float32).T).astype(BF)
    wsb = np.ascontiguousarray(np.asarray(Ws, dtype=np.float32).T).astype(BF)
    wxb = np.ascontiguousarray(np.asarray(Wx, dtype=np.float32).T).astype(BF)

    batch = nbat
    n_cores = _n_cores
    nb = _nb if _nb is not None else batch // n_cores
    assert nb * n_cores == batch
    assert batch % n_cores == 0

    nc = _get_program(nb, _repeat, _hw_loop)

    in_maps = []
    for c in range(n_cores):
        sl = slice(c * nb, (c + 1) * nb)
        in_maps.append({
            "L": L[sl], "I": I[sl],
            "wct": wct, "wcb": wcb, "wsb": wsb, "wxb": wxb,
        })
    res = bass_utils.run_bass_kernel_spmd(nc, in_maps, list(range(n_cores)))
    out = np.empty((batch, D, D), dtype=np.float32)
    for c in range(n_cores):
        out[c * nb:(c + 1) * nb] = res.results[c]["out"]
    return out
